# revision 3
# baseline (speedup 1.0000x reference)
"""CapsuleLayer (dynamic routing) Trainium2 kernel, v2.

Problem: x [64,1152,8] f32, W [1152,64,8,16] f32 ->
  u_hat = einsum('bid,iodc->bioc', x, W)
  3 routing iterations (softmax over o=64, weighted i-sum, squash, agreement)
  returns v [64,64,16] f32.

Sharding: data-parallel over batch, 8 batch elements per core x 8 cores.

v2 design (vs v1 424830ns):
  * u free order is (g, o*16+c)  [was (g, c*64+o)].
  * s-reduce via block-diag c weights: DVE builds ddiag[(isub,b),(b',gi,o)]
    = c[b,(g,isub),o] * [b==b'], PE streams raw u in 16-row matmuls
    (per (g,o)) accumulating s[b, o*16+c] in psum. Kills the smult pass
    (was 2x 73728 elems/part on DVE+Pool) and the tmps buffers.
  * s0 via W-as-weights: per (g, blk) matmul lhsT=W-tile-col-block,
    rhs=xdn column -> s0T[(o8,c), (blk,b)] psum: 4608 rows total
    (was 73728 rows re-streaming W against xdn).
  * s0T -> PE transpose -> 8 reorder DMAs -> s0r [8,1024]; all three
    boundaries then share the std squash.  v16 aliases vrep[0:8]; the
    16-fold isub replication is 4 partition-doubling DMAs (8->16->32->
    64->128) instead of 16 flat ones (HWDGE is ~625ns per DMA).
  * PE p-state: the cost model runs matmuls at 1.54/0.83/0.42 ns/row
    depending on ramp (3us of continuous busy => full speed).  Dummy
    filler matmuls (scratch psum, no sems) bridge PE idle gaps so the
    busy streak survives and real matmuls run at full rate.
  * Pool owns agr mult+tree for POOL_CHUNKS; DVE owns the rest plus all
    softmax/diag work.

Precision: fp16 inputs/u_hat/logits/exp/c, f32 psum accum + squash math.
"""

import numpy as np

NB = 8        # batch per core
NCORES = 8
G = 72        # i-groups of 16 in-capsules
CG = 4        # groups per routing chunk
CH = G // CG  # 18 chunks
O, C, D = 64, 16, 8
ISUB = 16     # in-caps per group
NWX = 6       # WX pipeline slots

POOL_CHUNKS = (2, 5, 8, 11, 14)
PP = len(POOL_CHUNKS)          # pool chunks per iter
DD = CH - PP

FILL_CHUNK = 9    # 512-row fillers after each routing chunk
FILL_B1 = 18      # fillers bridging squash (before the repl matmul)
FILL_B2 = 8       # fillers bridging the vrep evac (after repl)


def _is_pool(k):
    return k in POOL_CHUNKS


def _cnt_p(k):
    return sum(1 for j in POOL_CHUNKS if j <= k)


def _cnt_d(k):
    return (k + 1) - _cnt_p(k)


# phase-1 evac owner: ACT/DVE alternate; Pool (busy issuing SWDGE WX
# DMAs early on) takes the tail tiles
def _evac_owner(g):
    if g >= 64:
        return 'P'
    return 'A' if g % 2 == 0 else 'D'


def _dma_pool(g):
    # WX tiles issued via the Pool SWDGE queue (parallel to sync HWDGE).
    # Slots 4,5 are pool-exclusive: SWDGE sem updates require sems no
    # other queue touches.
    return g % NWX >= 4


def _evac_cnt(owner, g):
    return sum(1 for j in range(g + 1) if _evac_owner(j) == owner)


_cache = {}


def _build_program(paranoid=False):
    import concourse.bass as bass
    import concourse.mybir as mybir

    f16 = mybir.dt.float16
    f32 = mybir.dt.float32

    nc = bass.Bass('TRN2', target_bir_lowering=False, debug=False)

    # ---- DRAM I/O ----
    WX = nc.dram_tensor('WX', [G, 128, 1152], f16, kind='ExternalInput')
    XDN = nc.dram_tensor('XDN', [128, G * NB], f16, kind='ExternalInput')
    MASK = nc.dram_tensor('MASK', [128, NB * O], f16,
                          kind='ExternalInput')
    EYE = nc.dram_tensor('EYE', [128, 128], f16, kind='ExternalInput')
    REPS = nc.dram_tensor('REPS', [NB, 128], f16, kind='ExternalInput')
    EYEF = nc.dram_tensor('EYEF', [128, 128], f32, kind='ExternalInput')
    VOUT = nc.dram_tensor('VOUT', [NB, 1024], f32, kind='ExternalOutput')

    # ---- SBUF ----
    # u[(isub*NB+b), g*1024 + o*16 + c] fp16
    u = nc.alloc_sbuf_tensor('u', [128, G * 1024], f16)          # 144KB/part
    wxst = nc.alloc_sbuf_tensor('wxst', [128, NWX * 1152], f16)
    xdn = nc.alloc_sbuf_tensor('xdn', [128, G * NB], f16)
    eye = nc.alloc_sbuf_tensor('eye', [128, 128], f16)
    reps = nc.alloc_sbuf_tensor('reps', [NB, 128], f16)
    eyef = nc.alloc_sbuf_tensor('eyef', [128, 128], f32)
    mask = nc.alloc_sbuf_tensor('mask', [128, NB * O], f16)  # 1KB
    L = nc.alloc_sbuf_tensor('L', [128, G * O], f16)             # 9KB
    Ltmp = nc.alloc_sbuf_tensor('Ltmp', [128, CG * O], f16)
    Ltmp_p = nc.alloc_sbuf_tensor('Ltmp_p', [128, CG * O], f16)
    eb = [nc.alloc_sbuf_tensor('eb%d' % i, [128, CG * O], f16)
          for i in range(3)]
    cb = nc.alloc_sbuf_tensor('cb', [128, CG * O], f16)
    dd = [nc.alloc_sbuf_tensor('dd%d' % i, [128, NB * CG * O], f16)
          for i in range(2)]                                     # 4KB each
    tmpa = nc.alloc_sbuf_tensor('tmpa', [128, CG * 1024], f16)   # 8KB
    tmpp = nc.alloc_sbuf_tensor('tmpp', [128, CG * 1024], f16)   # 8KB
    Zb = nc.alloc_sbuf_tensor('Zb', [128, G], f32)
    zr = nc.alloc_sbuf_tensor('zr', [128, G], f32)
    # vrep[(isub,b), o*16+c]; rows 0:8 double as v16
    vrep = nc.alloc_sbuf_tensor('vrep', [128, 1024], f16)
    # boundary-0 path
    s0f = nc.alloc_sbuf_tensor('s0f', [128, 64], f32)
    pTs = nc.alloc_sbuf_tensor('pTs', [64, 128], f16)
    # std squash
    s2 = nc.alloc_sbuf_tensor('s2', [NB, 1024], f32)             # also vf
    sq = nc.alloc_sbuf_tensor('sq', [NB, O], f32)
    rr = nc.alloc_sbuf_tensor('rr', [NB, O], f32)
    q1 = nc.alloc_sbuf_tensor('q1', [NB, O], f32)
    q2 = nc.alloc_sbuf_tensor('q2', [NB, O], f32)
    ff = nc.alloc_sbuf_tensor('ff', [NB, O], f32)
    vf = s2  # s2's last read (the sq reduce) precedes the vf write

    # ---- PSUM ----
    pg0 = nc.alloc_psum_tensor('pg0', [128, 1024], f32)
    pg1 = nc.alloc_psum_tensor('pg1', [128, 1024], f32)
    pg2 = nc.alloc_psum_tensor('pg2', [128, 1024], f32)
    # ps128: phase1 uses cols 0:64 (s0T group, bank 0); routing uses
    # rows 0:8 as ps [8,1024].
    ps128 = nc.alloc_psum_tensor('ps128', [128, 1024], f32)
    pg = [pg0, pg1, pg2]
    ps = ps128.ap()[0:NB, :]
    s0T = ps128.ap()[:, 0:64]
    # boundary-0 f32 transpose target (pg1 is idle by then)
    pT = pg1.ap()[0:64, 0:128]

    AF = mybir.ActivationFunctionType
    AX = mybir.AxisListType

    sems = {}
    for name in ['d0', 'd0c', 'wxfree', 'pgsem', 'evsemA', 'evsemD', 'evsemP',
                 'ssem', 's0sem', 'qa', 'qb', 'qc',
                 's0fsem', 'pTsem', 'pTssem', 's0psem', 'repsem', 'v16sem',
                 'vfsem', 'vrsem', 'Lsem_d', 'Lsem_p', 'xsem', 'ebfree',
                 'dsem', 'pdone', 'dout']:
        sems[name] = nc.alloc_semaphore(name)
    wxsems = [nc.alloc_semaphore('wxs%d' % i) for i in range(NWX)]
    S = type('S', (), sems)

    def ap3(t, base, dims):
        a = t.ap()
        return bass.AP(a.tensor, base, [a.ap[0]] + [[s, n] for s, n in dims])

    def wxv(b, c0, c1):
        return wxst.ap()[:, b * 1152 + c0:b * 1152 + c1]

    def dr(eng):
        if paranoid:
            eng.drain()

    def filler(eng, out_ap, n, cols=512):
        """n dummy matmuls to keep the PE busy-streak (p-state) alive."""
        for _ in range(n):
            eng.matmul(out_ap, lhsT=eye.ap(),
                       rhs=mask.ap()[:, 0:cols], start=True, stop=True)

    def agr_block(eng, t, k, buf, ltbuf, lsem):
        """agr mult + c-tree + logit update for chunk k, iter t.

        u chunk per partition: [(1024, CG), (16, O), (1, C)].  Tree over
        c: offsets 8,4,2 then the final 16-lattice pair add writes the
        compact L slice [(O, CG), (1, O)].
        """
        ub = k * CG * 1024
        eng.tensor_mul(
            buf.ap(),
            ap3(u, ub, [(1024, CG), (1, 1024)]),
            ap3(vrep, 0, [(0, CG), (1, 1024)]))
        dr(eng)
        for off in (8, 4, 2):
            eng.tensor_add(
                ap3(buf, 0, [(1024, CG), (16, O), (1, off)]),
                ap3(buf, 0, [(1024, CG), (16, O), (1, off)]),
                ap3(buf, off, [(1024, CG), (16, O), (1, off)]))
            dr(eng)
        lsl = ap3(L, k * CG * O, [(O, CG), (1, O)])
        t3a = ap3(buf, 0, [(1024, CG), (16, O)])
        t3b = ap3(buf, 1, [(1024, CG), (16, O)])
        if t == 0:
            eng.tensor_add(lsl, t3a, t3b).then_inc(lsem, 1)
        else:
            eng.tensor_add(ltbuf.ap(), t3a, t3b)
            dr(eng)
            eng.tensor_add(lsl, lsl, ltbuf.ap()).then_inc(lsem, 1)
        dr(eng)

    with nc.allow_low_precision(reason='fp16 validated vs f32 ref'), \
         nc.Block() as block:

        # ---------------- SYNC: main DMA queue ----------------
        @block.sync
        def _(eng):
            eng.dma_start(xdn.ap(), XDN.ap()).then_inc(S.d0, 16)
            for g in range(G):
                s = g % NWX
                if s >= 4:
                    continue
                if g >= NWX:
                    eng.wait_ge(S.wxfree, g - NWX + 1)
                eng.dma_start(wxv(s, 0, 1152), WX.ap()[g]) \
                   .then_inc(wxsems[s], 16)
            eng.wait_ge(S.vfsem, 1)
            eng.dma_start(VOUT.ap(), vf.ap()).then_inc(S.dout, 16)

        # ---------------- PE ----------------
        @block.tensor
        def _(eng):
            # phase 1: u_hat tiles + s0T accumulation (+ fillers)
            for g in range(G):
                b = g % NWX
                eng.wait_ge(wxsems[b], 16 * (g // NWX + 1))
                if g >= 3:
                    gp = g - 3
                    own = _evac_owner(gp)
                    sem = {'A': S.evsemA, 'D': S.evsemD, 'P': S.evsemP}[own]
                    eng.wait_ge(sem, _evac_cnt(own, gp))
                eng.matmul(pg[g % 3].ap()[:, 0:512],
                           lhsT=wxv(b, 1024, 1152),
                           rhs=wxv(b, 0, 512), start=True, stop=True)
                eng.matmul(pg[g % 3].ap()[:, 512:1024],
                           lhsT=wxv(b, 1024, 1152),
                           rhs=wxv(b, 512, 1024),
                           start=True, stop=True).then_inc(S.pgsem, 1)
                if g == 0:
                    eng.wait_ge(S.d0, 16)
                for blk in range(8):
                    mm = eng.matmul(
                        s0T[:, blk * NB:(blk + 1) * NB],
                        lhsT=wxv(b, blk * 128, (blk + 1) * 128),
                        rhs=xdn.ap()[:, g * NB:(g + 1) * NB],
                        start=(g == 0 and blk == 0),
                        stop=(g == G - 1 and blk == 7))
                mm.then_inc(S.wxfree, 1)
                if g == G - 1:
                    eng.maybe_drain_then_inc((S.s0sem, 1))
            # boundary 0: transpose s0f -> pT (a pg1 region; wait until
            # all evacs have drained pg before overwriting)
            eng.wait_ge(S.evsemA, _evac_cnt('A', G - 1))
            eng.wait_ge(S.evsemD, _evac_cnt('D', G - 1))
            eng.wait_ge(S.evsemP, _evac_cnt('P', G - 1))
            eng.wait_ge(S.d0c, 64)
            eng.wait_ge(S.s0fsem, 1)
            eng.transpose(pT, s0f.ap(), eyef.ap())
            eng.maybe_drain_then_inc((S.pTsem, 1))
            # reorder pTs -> ps[b, blk*128+j] via identity-slice matmuls
            eng.wait_ge(S.pTssem, 1)
            for blk in range(8):
                eng.matmul(ps[:, blk * 128:(blk + 1) * 128],
                           lhsT=eye.ap()[0:64, blk * NB:(blk + 1) * NB],
                           rhs=pTs.ap(),
                           start=(blk % 4 == 0), stop=(blk % 4 == 3))
            eng.maybe_drain_then_inc((S.s0psem, 1))
            filler(eng, pg0.ap()[:, 0:512], FILL_B1)
            # routing: s-reduce via diag-c weights, 16-row matmuls
            for t in range(2):
                # vrep = replicate v16 (vrep rows 0:8) across isub via PE
                eng.wait_ge(S.v16sem, t + 1)
                for h in range(2):
                    mm = eng.matmul(
                        pg1.ap()[:, h * 512:(h + 1) * 512],
                        lhsT=reps.ap(),
                        rhs=vrep.ap()[0:NB, h * 512:(h + 1) * 512],
                        start=True, stop=True)
                mm.then_inc(S.repsem, 1)
                filler(eng, pg0.ap()[:, 0:512], FILL_B2)
                for k in range(CH):
                    n = t * CH + k
                    eng.wait_ge(S.dsem, n + 1)
                    for gi in range(CG):
                        gg = k * CG + gi
                        for o in range(O):
                            mm = eng.matmul(
                                ps[:, o * C:(o + 1) * C],
                                lhsT=ap3(dd[n % 2], gi * O + o,
                                         [(CG * O, NB)]),
                                rhs=u.ap()[:, gg * 1024 + o * C:
                                           gg * 1024 + (o + 1) * C],
                                start=(k == 0 and gi == 0 and o % 32 == 0),
                                stop=(k == CH - 1 and gi == CG - 1
                                      and o % 32 == 31))
                    mm.then_inc(S.pdone, 1)
                    if k == CH - 1:
                        eng.maybe_drain_then_inc((S.ssem, 1))
                        if t == 0:
                            filler(eng, pg0.ap()[:, 0:512], FILL_B1)
                    else:
                        filler(eng, pg0.ap()[:, 0:512], FILL_CHUNK)

        # ---------------- ACT (scalar) ----------------
        @block.scalar
        def _(eng):
            # phase 1: WX loads for slots 4,5 (second HWDGE queue) and
            # evac share, interleaved in g-order
            consts_done = [False]

            def act_consts():
                if consts_done[0]:
                    return
                consts_done[0] = True
                eng.dma_start(eye.ap(), EYE.ap()).then_inc(S.d0c, 16)
                eng.dma_start(reps.ap(), REPS.ap()).then_inc(S.d0c, 16)
                eng.dma_start(eyef.ap(), EYEF.ap()).then_inc(S.d0c, 16)
                eng.dma_start(mask.ap(), MASK.ap()).then_inc(S.d0c, 16)

            for g in range(G):
                s = g % NWX
                if s >= 4:
                    if g >= NWX:
                        eng.wait_ge(S.wxfree, g - NWX + 1)
                    eng.dma_start(wxv(s, 0, 1152), WX.ap()[g]) \
                       .then_inc(wxsems[s], 16)
                    if g > 40:
                        act_consts()
                if _evac_owner(g) == 'A':
                    eng.wait_ge(S.pgsem, g + 1)
                    eng.activation(ap3(u, g * 1024, [(1, 1024)]),
                                   pg[g % 3].ap(), AF.Copy) \
                       .then_inc(S.evsemA, 1)
            act_consts()
            # boundary 0: s0T -> fp16, then pT -> pTs
            eng.wait_ge(S.s0sem, 1)
            eng.activation(s0f.ap(), s0T, AF.Copy).then_inc(S.s0fsem, 1)
            eng.wait_ge(S.pTsem, 1)
            eng.activation(pTs.ap(), pT, AF.Copy).then_inc(S.pTssem, 1)
            # boundary-0 squash (s0 sits in ps)
            eng.wait_ge(S.s0psem, 1)
            eng.activation(s2.ap(), ps, AF.Square).then_inc(S.qa, 1)
            eng.wait_ge(S.qb, 1)
            eng.activation(rr.ap(), sq.ap(), AF.Sqrt).then_inc(S.qc, 1)
            # replicate: evac repl-matmul result into vrep rows 8:128
            eng.wait_ge(S.repsem, 1)
            eng.activation(vrep.ap(), pg1.ap(), AF.Copy).then_inc(S.vrsem, 1)
            # routing exp + squash at boundaries 1,2
            for t in range(2):
                for k in range(CH):
                    n = t * CH + k
                    if _is_pool(k):
                        eng.wait_ge(S.Lsem_p, t * PP + _cnt_p(k))
                    else:
                        eng.wait_ge(S.Lsem_d, t * DD + _cnt_d(k))
                    if n >= 3:
                        eng.wait_ge(S.ebfree, n - 2)
                    for gi in range(CG):
                        a = eng.activation(
                            ap3(eb[n % 3], gi * O, [(1, O)]),
                            ap3(L, (k * CG + gi) * O, [(1, O)]),
                            AF.Exp,
                            accum_out=ap3(Zb, k * CG + gi, [(1, 1)]))
                    a.then_inc(S.xsem, 1)
                eng.wait_ge(S.ssem, t + 1)
                eng.activation(s2.ap(), ps, AF.Square).then_inc(S.qa, 1)
                eng.wait_ge(S.qb, t + 2)
                eng.activation(rr.ap(), sq.ap(), AF.Sqrt).then_inc(S.qc, 1)
                if t == 0:
                    eng.wait_ge(S.repsem, 2)
                    eng.activation(vrep.ap(), pg1.ap(), AF.Copy) \
                       .then_inc(S.vrsem, 1)

        # ---------------- DVE (vector) ----------------
        @block.vector
        def _(eng):
            def softmax_diag(t, j):
                """softmax normalize chunk j + build diag weights."""
                nj = t * CH + j
                if nj == 0:
                    eng.wait_ge(S.d0c, 64)
                eng.wait_ge(S.xsem, nj + 1)
                eng.reciprocal(ap3(zr, j * CG, [(1, CG)]),
                               ap3(Zb, j * CG, [(1, CG)]))
                dr(eng)
                eng.tensor_mul(cb.ap(),
                               ap3(eb[nj % 3], 0, [(O, CG), (1, O)]),
                               ap3(zr, j * CG, [(1, CG), (0, O)])) \
                   .then_inc(S.ebfree, 1)
                dr(eng)
                if nj >= 2:
                    eng.wait_ge(S.pdone, nj - 1)
                eng.tensor_mul(
                    ap3(dd[nj % 2], 0, [(CG * O, NB), (O, CG), (1, O)]),
                    ap3(cb, 0, [(0, NB), (O, CG), (1, O)]),
                    ap3(mask, 0, [(O, NB), (0, CG), (1, O)])) \
                    .then_inc(S.dsem, 1)
                dr(eng)

            def squash_std(bi, src, out, outsem):
                """squash: src [8,1024] (o,c) -> out = src * ff_bcast."""
                eng.wait_ge(S.qa, bi + 1)
                eng.reduce_sum(sq.ap(),
                               ap3(s2, 0, [(C, O), (1, C)]),
                               axis=AX.X).then_inc(S.qb, 1)
                dr(eng)
                eng.wait_ge(S.qc, bi + 1)
                eng.tensor_scalar_add(q1.ap(), sq.ap(), 1.0)
                eng.tensor_scalar_add(q2.ap(), rr.ap(), 1e-8)
                dr(eng)
                eng.tensor_mul(q1.ap(), q1.ap(), q2.ap())
                dr(eng)
                eng.reciprocal(q2.ap(), q1.ap())
                dr(eng)
                eng.tensor_mul(ff.ap(), sq.ap(), q2.ap())
                dr(eng)
                fb = ap3(ff, 0, [(1, O), (0, C)])
                eng.tensor_mul(out, src, fb).then_inc(outsem, 1)

            # phase 1 evac share
            for g in range(G):
                if _evac_owner(g) != 'D':
                    continue
                eng.wait_ge(S.pgsem, g + 1)
                eng.tensor_copy(ap3(u, g * 1024, [(1, 1024)]),
                                pg[g % 3].ap()).then_inc(S.evsemD, 1)
            # boundary-0 squash -> vrep[0:8]
            squash_std(0, ps, vrep.ap()[0:NB, :], S.v16sem)
            # routing: agr(k) then softmax(k-1) to hide ACT exp latency
            for t in range(2):
                for k in range(CH):
                    if t == 0:
                        glast = (k + 1) * CG - 1
                        for own, sem in (('A', S.evsemA), ('D', S.evsemD),
                                         ('P', S.evsemP)):
                            cnt = _evac_cnt(own, glast)
                            if cnt:
                                eng.wait_ge(sem, cnt)
                    if k == 0:
                        eng.wait_ge(S.vrsem, t + 1)
                    if not _is_pool(k):
                        agr_block(eng, t, k, tmpa, Ltmp, S.Lsem_d)
                    if k > 1:
                        softmax_diag(t, k - 2)
                softmax_diag(t, CH - 2)
                softmax_diag(t, CH - 1)
                if t == 0:
                    squash_std(1, ps, vrep.ap()[0:NB, :], S.v16sem)
                else:
                    squash_std(2, ps, vf.ap(), S.vfsem)

        # ---------------- GpSimd (pool) ----------------
        @block.gpsimd
        def _(eng):
            for g in range(G):
                if _evac_owner(g) != 'P':
                    continue
                eng.wait_ge(S.pgsem, g + 1)
                eng.tensor_copy(ap3(u, g * 1024, [(1, 1024)]),
                                pg[g % 3].ap()).then_inc(S.evsemP, 1)
            # routing: agr mult+tree for pool chunks
            for t in range(2):
                for k in POOL_CHUNKS:
                    if t == 0:
                        glast = (k + 1) * CG - 1
                        for own, sem in (('A', S.evsemA), ('D', S.evsemD),
                                         ('P', S.evsemP)):
                            cnt = _evac_cnt(own, glast)
                            if cnt:
                                eng.wait_ge(sem, cnt)
                    eng.wait_ge(S.vrsem, t + 1)
                    agr_block(eng, t, k, tmpp, Ltmp_p, S.Lsem_p)

    return nc


def _preprocess(x, W):
    """Host-side repack (fp16 casts + layout) -> per-core input maps."""
    f16 = np.float16
    # W tiles: [g, (isub*8+d), (o*16+c)]
    Wt = np.ascontiguousarray(
        W.reshape(G, ISUB, O, D, C).transpose(0, 1, 3, 2, 4)
        .reshape(G, 128, 1024)).astype(f16)
    eyem = np.eye(128).astype(f16)
    # REPS[b, (isub,b')] = 1 if b == b' (replicates v16 over isub via PE)
    repsm = np.zeros((NB, 128), f16)
    for col in range(128):
        repsm[col % NB, col] = 1.0
    # MASK[(isub,b), (b', o)] = 1 if b==b'
    maskm = np.zeros((128, NB * O), f16)
    for p in range(128):
        b = p % NB
        maskm[p, b * O:(b + 1) * O] = 1.0
    in_maps = []
    for core in range(NCORES):
        xc = x[core * NB:(core + 1) * NB]            # [8, 1152, 8]
        xr = xc.reshape(NB, G, ISUB, D)              # (b, g, isub, d)
        xbd = np.zeros((G, 128, 128), f16)
        for isub in range(ISUB):
            xbd[:, isub * D:(isub + 1) * D, isub * NB:(isub + 1) * NB] = \
                xr[:, :, isub, :].transpose(1, 2, 0)  # (g, d, b)
        WXc = np.concatenate([Wt, xbd], axis=2)      # [72, 128, 1152]
        xdn = np.ascontiguousarray(
            (xr / 64.0).transpose(2, 3, 1, 0).reshape(128, G * NB)).astype(f16)
        in_maps.append({'WX': WXc, 'XDN': xdn, 'MASK': maskm, 'EYE': eyem,
                        'REPS': repsm, 'EYEF': eyem.astype(np.float32)})
    return in_maps


def _postprocess(results):
    out = np.empty((NCORES * NB, O, C), np.float32)
    for core in range(NCORES):
        vo = results[core]['VOUT']                   # [8, 1024] = (o, c)
        out[core * NB:(core + 1) * NB] = vo.reshape(NB, O, C)
    return out


def kernel(x, W):
    from concourse.bass_utils import run_bass_kernel_spmd
    x = np.asarray(x, np.float32)
    W = np.asarray(W, np.float32)
    if 'nc' not in _cache:
        _cache['nc'] = _build_program(paranoid=True)
    in_maps = _preprocess(x, W)
    res = run_bass_kernel_spmd(_cache['nc'], in_maps,
                               core_ids=list(range(NCORES)))
    return _postprocess(res.results)


def kernel_sim(x, W, core=0):
    """CoreSim single-core check: returns v for that core's 8 batch rows."""
    from concourse import bass_interp
    x = np.asarray(x, np.float32)
    W = np.asarray(W, np.float32)
    if 'nc_sim' not in _cache:
        _cache['nc_sim'] = _build_program(paranoid=True)
    in_maps = _preprocess(x, W)
    sim = bass_interp.CoreSim(_cache['nc_sim'])
    for name, arr in in_maps[core].items():
        sim.tensor(name)[:] = arr
    sim.simulate()
    vo = np.asarray(sim.tensor('VOUT'))
    return vo.reshape(NB, O, C)


# revision 4
# speedup vs baseline: 1.2284x; 1.2284x over previous
"""CapsuleLayer (dynamic routing) Trainium2 kernel, v2.

Problem: x [64,1152,8] f32, W [1152,64,8,16] f32 ->
  u_hat = einsum('bid,iodc->bioc', x, W)
  3 routing iterations (softmax over o=64, weighted i-sum, squash, agreement)
  returns v [64,64,16] f32.

Sharding: data-parallel over batch, 8 batch elements per core x 8 cores.

v2 design (vs v1 424830ns):
  * u free order is (g, o*16+c)  [was (g, c*64+o)].
  * s-reduce via block-diag c weights: DVE builds ddiag[(isub,b),(b',gi,o)]
    = c[b,(g,isub),o] * [b==b'], PE streams raw u in 16-row matmuls
    (per (g,o)) accumulating s[b, o*16+c] in psum. Kills the smult pass
    (was 2x 73728 elems/part on DVE+Pool) and the tmps buffers.
  * s0 via W-as-weights: per (g, blk) matmul lhsT=W-tile-col-block,
    rhs=xdn column -> s0T[(o8,c), (blk,b)] psum: 4608 rows total
    (was 73728 rows re-streaming W against xdn).
  * s0T -> PE transpose -> 8 reorder DMAs -> s0r [8,1024]; all three
    boundaries then share the std squash.  v16 aliases vrep[0:8]; the
    16-fold isub replication is 4 partition-doubling DMAs (8->16->32->
    64->128) instead of 16 flat ones (HWDGE is ~625ns per DMA).
  * PE p-state: the cost model runs matmuls at 1.54/0.83/0.42 ns/row
    depending on ramp (3us of continuous busy => full speed).  Dummy
    filler matmuls (scratch psum, no sems) bridge PE idle gaps so the
    busy streak survives and real matmuls run at full rate.
  * Pool owns agr mult+tree for POOL_CHUNKS; DVE owns the rest plus all
    softmax/diag work.

Precision: fp16 inputs/u_hat/logits/exp/c, f32 psum accum + squash math.
"""

import numpy as np

NB = 8        # batch per core
NCORES = 8
G = 72        # i-groups of 16 in-capsules
CG = 4        # groups per routing chunk
CH = G // CG  # 18 chunks
O, C, D = 64, 16, 8
ISUB = 16     # in-caps per group
NWX = 6       # WX pipeline slots

POOL_CHUNKS = (2, 5, 8, 11, 14)
PP = len(POOL_CHUNKS)          # pool chunks per iter
DD = CH - PP

FILL_CHUNK = 9    # 512-row fillers after each routing chunk
FILL_B1 = 18      # fillers bridging squash (before the repl matmul)
FILL_B2 = 8       # fillers bridging the vrep evac (after repl)


def _is_pool(k):
    return k in POOL_CHUNKS


def _cnt_p(k):
    return sum(1 for j in POOL_CHUNKS if j <= k)


def _cnt_d(k):
    return (k + 1) - _cnt_p(k)


# phase-1 evac owner: ACT/DVE alternate (GpSimd has no PSUM port on HW)
def _evac_owner(g):
    return 'A' if g % 2 == 0 else 'D'


def _dma_pool(g):
    # WX tiles issued via the Pool SWDGE queue (parallel to sync HWDGE).
    # Slots 4,5 are pool-exclusive: SWDGE sem updates require sems no
    # other queue touches.
    return g % NWX >= 4


def _evac_cnt(owner, g):
    return sum(1 for j in range(g + 1) if _evac_owner(j) == owner)


_cache = {}


def _build_program(paranoid=False):
    import concourse.bass as bass
    import concourse.mybir as mybir

    f16 = mybir.dt.float16
    f32 = mybir.dt.float32

    nc = bass.Bass('TRN2', target_bir_lowering=False, debug=False)

    # ---- DRAM I/O ----
    WX = nc.dram_tensor('WX', [G, 128, 1152], f16, kind='ExternalInput')
    XDN = nc.dram_tensor('XDN', [128, G * NB], f16, kind='ExternalInput')
    MASK = nc.dram_tensor('MASK', [128, NB * O], f16,
                          kind='ExternalInput')
    EYE = nc.dram_tensor('EYE', [128, 128], f16, kind='ExternalInput')
    REPS = nc.dram_tensor('REPS', [NB, 128], f16, kind='ExternalInput')
    EYEF = nc.dram_tensor('EYEF', [128, 128], f32, kind='ExternalInput')
    VOUT = nc.dram_tensor('VOUT', [NB, 1024], f32, kind='ExternalOutput')

    # ---- SBUF ----
    # u[(isub*NB+b), g*1024 + o*16 + c] fp16
    u = nc.alloc_sbuf_tensor('u', [128, G * 1024], f16)          # 144KB/part
    wxst = nc.alloc_sbuf_tensor('wxst', [128, NWX * 1152], f16)
    xdn = nc.alloc_sbuf_tensor('xdn', [128, G * NB], f16)
    eye = nc.alloc_sbuf_tensor('eye', [128, 128], f16)
    reps = nc.alloc_sbuf_tensor('reps', [NB, 128], f16)
    eyef = nc.alloc_sbuf_tensor('eyef', [128, 128], f32)
    mask = nc.alloc_sbuf_tensor('mask', [128, NB * O], f16)  # 1KB
    L = nc.alloc_sbuf_tensor('L', [128, G * O], f16)             # 9KB
    Ltmp = nc.alloc_sbuf_tensor('Ltmp', [128, CG * O], f16)
    Ltmp_p = nc.alloc_sbuf_tensor('Ltmp_p', [128, CG * O], f16)
    eb = [nc.alloc_sbuf_tensor('eb%d' % i, [128, CG * O], f16)
          for i in range(3)]
    cb = nc.alloc_sbuf_tensor('cb', [128, CG * O], f16)
    dd = [nc.alloc_sbuf_tensor('dd%d' % i, [128, NB * CG * O], f16)
          for i in range(2)]                                     # 4KB each
    tmpa = nc.alloc_sbuf_tensor('tmpa', [128, CG * 1024], f16)   # 8KB
    tmpp = nc.alloc_sbuf_tensor('tmpp', [128, CG * 1024], f16)   # 8KB
    Zb = nc.alloc_sbuf_tensor('Zb', [128, G], f32)
    zr = nc.alloc_sbuf_tensor('zr', [128, G], f32)
    # vrep[(isub,b), o*16+c]; rows 0:8 double as v16
    vrep = nc.alloc_sbuf_tensor('vrep', [128, 1024], f16)
    # boundary-0 path
    s0f = nc.alloc_sbuf_tensor('s0f', [128, 64], f32)
    pTs = nc.alloc_sbuf_tensor('pTs', [64, 128], f16)
    # std squash
    s2 = nc.alloc_sbuf_tensor('s2', [NB, 1024], f32)             # also vf
    sq = nc.alloc_sbuf_tensor('sq', [NB, O], f32)
    rr = nc.alloc_sbuf_tensor('rr', [NB, O], f32)
    q1 = nc.alloc_sbuf_tensor('q1', [NB, O], f32)
    q2 = nc.alloc_sbuf_tensor('q2', [NB, O], f32)
    ff = nc.alloc_sbuf_tensor('ff', [NB, O], f32)
    vf = s2  # s2's last read (the sq reduce) precedes the vf write

    # ---- PSUM ----
    pg0 = nc.alloc_psum_tensor('pg0', [128, 1024], f32)
    pg1 = nc.alloc_psum_tensor('pg1', [128, 1024], f32)
    pg2 = nc.alloc_psum_tensor('pg2', [128, 1024], f32)
    # ps128: phase1 uses cols 0:64 (s0T group, bank 0); routing uses
    # rows 0:8 as ps [8,1024].
    ps128 = nc.alloc_psum_tensor('ps128', [128, 1024], f32)
    pg = [pg0, pg1, pg2]
    ps = ps128.ap()[0:NB, :]
    s0T = ps128.ap()[:, 0:64]
    # boundary-0 f32 transpose target (pg1 is idle by then)
    pT = pg1.ap()[0:64, 0:128]

    AF = mybir.ActivationFunctionType
    AX = mybir.AxisListType

    sems = {}
    for name in ['d0', 'd0c', 'wxfree', 'pgsem', 'evsemA', 'evsemD', 'evsemP',
                 'ssem', 's0sem', 'qa', 'qb', 'qc',
                 's0fsem', 'pTsem', 'pTssem', 's0psem', 'repsem', 'v16sem',
                 'vfsem', 'vrsem', 'Lsem_d', 'Lsem_p', 'xsem', 'ebfree',
                 'dsem', 'pdone', 'dout']:
        sems[name] = nc.alloc_semaphore(name)
    wxsems = [nc.alloc_semaphore('wxs%d' % i) for i in range(NWX)]
    S = type('S', (), sems)

    def ap3(t, base, dims):
        a = t.ap()
        return bass.AP(a.tensor, base, [a.ap[0]] + [[s, n] for s, n in dims])

    def wxv(b, c0, c1):
        return wxst.ap()[:, b * 1152 + c0:b * 1152 + c1]

    def dr(eng):
        if paranoid:
            eng.drain()

    def filler(eng, out_ap, n, cols=512):
        """n dummy matmuls to keep the PE busy-streak (p-state) alive."""
        for _ in range(n):
            eng.matmul(out_ap, lhsT=eye.ap(),
                       rhs=mask.ap()[:, 0:cols], start=True, stop=True)

    def agr_block(eng, t, k, buf, ltbuf, lsem):
        """agr mult + c-tree + logit update for chunk k, iter t.

        u chunk per partition: [(1024, CG), (16, O), (1, C)].  Tree over
        c: offsets 8,4,2 then the final 16-lattice pair add writes the
        compact L slice [(O, CG), (1, O)].
        """
        ub = k * CG * 1024
        eng.tensor_mul(
            buf.ap(),
            ap3(u, ub, [(1024, CG), (1, 1024)]),
            ap3(vrep, 0, [(0, CG), (1, 1024)]))
        dr(eng)
        for off in (8, 4, 2):
            eng.tensor_add(
                ap3(buf, 0, [(1024, CG), (16, O), (1, off)]),
                ap3(buf, 0, [(1024, CG), (16, O), (1, off)]),
                ap3(buf, off, [(1024, CG), (16, O), (1, off)]))
            dr(eng)
        lsl = ap3(L, k * CG * O, [(O, CG), (1, O)])
        t3a = ap3(buf, 0, [(1024, CG), (16, O)])
        t3b = ap3(buf, 1, [(1024, CG), (16, O)])
        if t == 0:
            eng.tensor_add(lsl, t3a, t3b).then_inc(lsem, 1)
        else:
            eng.tensor_add(ltbuf.ap(), t3a, t3b)
            dr(eng)
            eng.tensor_add(lsl, lsl, ltbuf.ap()).then_inc(lsem, 1)
        dr(eng)

    with nc.allow_low_precision(reason='fp16 validated vs f32 ref'), \
         nc.Block() as block:

        # ---------------- SYNC: main DMA queue ----------------
        @block.sync
        def _(eng):
            eng.dma_start(xdn.ap(), XDN.ap()).then_inc(S.d0, 16)
            for g in range(G):
                s = g % NWX
                if s >= 4:
                    continue
                if g >= NWX:
                    eng.wait_ge(S.wxfree, g - NWX + 1)
                eng.dma_start(wxv(s, 0, 1152), WX.ap()[g]) \
                   .then_inc(wxsems[s], 16)
            eng.wait_ge(S.vfsem, 1)
            eng.dma_start(VOUT.ap(), vf.ap()).then_inc(S.dout, 16)

        # ---------------- PE ----------------
        @block.tensor
        def _(eng):
            # phase 1: u_hat tiles + s0T accumulation (+ fillers)
            for g in range(G):
                b = g % NWX
                eng.wait_ge(wxsems[b], 16 * (g // NWX + 1))
                if g >= 3:
                    gp = g - 3
                    own = _evac_owner(gp)
                    sem = {'A': S.evsemA, 'D': S.evsemD, 'P': S.evsemP}[own]
                    eng.wait_ge(sem, _evac_cnt(own, gp))
                eng.matmul(pg[g % 3].ap()[:, 0:512],
                           lhsT=wxv(b, 1024, 1152),
                           rhs=wxv(b, 0, 512), start=True, stop=True)
                eng.matmul(pg[g % 3].ap()[:, 512:1024],
                           lhsT=wxv(b, 1024, 1152),
                           rhs=wxv(b, 512, 1024),
                           start=True, stop=True).then_inc(S.pgsem, 1)
                if g == 0:
                    eng.wait_ge(S.d0, 16)
                for blk in range(8):
                    mm = eng.matmul(
                        s0T[:, blk * NB:(blk + 1) * NB],
                        lhsT=wxv(b, blk * 128, (blk + 1) * 128),
                        rhs=xdn.ap()[:, g * NB:(g + 1) * NB],
                        start=(g == 0 and blk == 0),
                        stop=(g == G - 1 and blk == 7))
                mm.then_inc(S.wxfree, 1)
                if g == G - 1:
                    eng.maybe_drain_then_inc((S.s0sem, 1))
            # boundary 0: transpose s0f -> pT (a pg1 region; wait until
            # all evacs have drained pg before overwriting)
            eng.wait_ge(S.evsemA, _evac_cnt('A', G - 1))
            eng.wait_ge(S.evsemD, _evac_cnt('D', G - 1))
            eng.wait_ge(S.evsemP, _evac_cnt('P', G - 1))
            eng.wait_ge(S.d0c, 64)
            eng.wait_ge(S.s0fsem, 1)
            eng.transpose(pT, s0f.ap(), eyef.ap())
            eng.maybe_drain_then_inc((S.pTsem, 1))
            # reorder pTs -> ps[b, blk*128+j] via identity-slice matmuls
            eng.wait_ge(S.pTssem, 1)
            for blk in range(8):
                eng.matmul(ps[:, blk * 128:(blk + 1) * 128],
                           lhsT=eye.ap()[0:64, blk * NB:(blk + 1) * NB],
                           rhs=pTs.ap(),
                           start=(blk % 4 == 0), stop=(blk % 4 == 3))
            eng.maybe_drain_then_inc((S.s0psem, 1))
            filler(eng, pg0.ap()[:, 0:512], FILL_B1)
            # routing: s-reduce via diag-c weights, 16-row matmuls
            for t in range(2):
                # vrep = replicate v16 (vrep rows 0:8) across isub via PE
                eng.wait_ge(S.v16sem, t + 1)
                for h in range(2):
                    mm = eng.matmul(
                        pg1.ap()[:, h * 512:(h + 1) * 512],
                        lhsT=reps.ap(),
                        rhs=vrep.ap()[0:NB, h * 512:(h + 1) * 512],
                        start=True, stop=True)
                mm.then_inc(S.repsem, 1)
                filler(eng, pg0.ap()[:, 0:512], FILL_B2)
                for k in range(CH):
                    n = t * CH + k
                    eng.wait_ge(S.dsem, n + 1)
                    for gi in range(CG):
                        gg = k * CG + gi
                        for o in range(O):
                            mm = eng.matmul(
                                ps[:, o * C:(o + 1) * C],
                                lhsT=ap3(dd[n % 2], gi * O + o,
                                         [(CG * O, NB)]),
                                rhs=u.ap()[:, gg * 1024 + o * C:
                                           gg * 1024 + (o + 1) * C],
                                start=(k == 0 and gi == 0 and o % 32 == 0),
                                stop=(k == CH - 1 and gi == CG - 1
                                      and o % 32 == 31))
                    mm.then_inc(S.pdone, 1)
                    if k == CH - 1:
                        eng.maybe_drain_then_inc((S.ssem, 1))
                        if t == 0:
                            filler(eng, pg0.ap()[:, 0:512], FILL_B1)
                    else:
                        filler(eng, pg0.ap()[:, 0:512], FILL_CHUNK)

        # ---------------- ACT (scalar) ----------------
        @block.scalar
        def _(eng):
            # phase 1 evac share
            for g in range(G):
                if _evac_owner(g) != 'A':
                    continue
                eng.wait_ge(S.pgsem, g + 1)
                eng.activation(ap3(u, g * 1024, [(1, 1024)]),
                               pg[g % 3].ap(), AF.Copy).then_inc(S.evsemA, 1)
            # boundary 0: s0T -> fp16, then pT -> pTs
            eng.wait_ge(S.s0sem, 1)
            eng.activation(s0f.ap(), s0T, AF.Copy).then_inc(S.s0fsem, 1)
            eng.wait_ge(S.pTsem, 1)
            eng.activation(pTs.ap(), pT, AF.Copy).then_inc(S.pTssem, 1)
            # boundary-0 squash (s0 sits in ps)
            eng.wait_ge(S.s0psem, 1)
            eng.activation(s2.ap(), ps, AF.Square).then_inc(S.qa, 1)
            eng.wait_ge(S.qb, 1)
            eng.activation(rr.ap(), sq.ap(), AF.Sqrt).then_inc(S.qc, 1)
            # replicate: evac repl-matmul result into vrep rows 8:128
            eng.wait_ge(S.repsem, 1)
            eng.activation(vrep.ap(), pg1.ap(), AF.Copy).then_inc(S.vrsem, 1)
            # routing exp + squash at boundaries 1,2
            for t in range(2):
                for k in range(CH):
                    n = t * CH + k
                    if _is_pool(k):
                        eng.wait_ge(S.Lsem_p, t * PP + _cnt_p(k))
                    else:
                        eng.wait_ge(S.Lsem_d, t * DD + _cnt_d(k))
                    if n >= 3:
                        eng.wait_ge(S.ebfree, n - 2)
                    for gi in range(CG):
                        a = eng.activation(
                            ap3(eb[n % 3], gi * O, [(1, O)]),
                            ap3(L, (k * CG + gi) * O, [(1, O)]),
                            AF.Exp,
                            accum_out=ap3(Zb, k * CG + gi, [(1, 1)]))
                    a.then_inc(S.xsem, 1)
                eng.wait_ge(S.ssem, t + 1)
                eng.activation(s2.ap(), ps, AF.Square).then_inc(S.qa, 1)
                eng.wait_ge(S.qb, t + 2)
                eng.activation(rr.ap(), sq.ap(), AF.Sqrt).then_inc(S.qc, 1)
                if t == 0:
                    eng.wait_ge(S.repsem, 2)
                    eng.activation(vrep.ap(), pg1.ap(), AF.Copy) \
                       .then_inc(S.vrsem, 1)

        # ---------------- DVE (vector) ----------------
        @block.vector
        def _(eng):
            def softmax_diag(t, j):
                """softmax normalize chunk j + build diag weights."""
                nj = t * CH + j
                if nj == 0:
                    eng.wait_ge(S.d0c, 64)
                eng.wait_ge(S.xsem, nj + 1)
                eng.reciprocal(ap3(zr, j * CG, [(1, CG)]),
                               ap3(Zb, j * CG, [(1, CG)]))
                dr(eng)
                eng.tensor_mul(cb.ap(),
                               ap3(eb[nj % 3], 0, [(O, CG), (1, O)]),
                               ap3(zr, j * CG, [(1, CG), (0, O)])) \
                   .then_inc(S.ebfree, 1)
                dr(eng)
                if nj >= 2:
                    eng.wait_ge(S.pdone, nj - 1)
                eng.tensor_mul(
                    ap3(dd[nj % 2], 0, [(CG * O, NB), (O, CG), (1, O)]),
                    ap3(cb, 0, [(0, NB), (O, CG), (1, O)]),
                    ap3(mask, 0, [(O, NB), (0, CG), (1, O)])) \
                    .then_inc(S.dsem, 1)
                dr(eng)

            def squash_std(bi, src, out, outsem):
                """squash: src [8,1024] (o,c) -> out = src * ff_bcast."""
                eng.wait_ge(S.qa, bi + 1)
                eng.reduce_sum(sq.ap(),
                               ap3(s2, 0, [(C, O), (1, C)]),
                               axis=AX.X).then_inc(S.qb, 1)
                dr(eng)
                eng.wait_ge(S.qc, bi + 1)
                eng.tensor_scalar_add(q1.ap(), sq.ap(), 1.0)
                eng.tensor_scalar_add(q2.ap(), rr.ap(), 1e-8)
                dr(eng)
                eng.tensor_mul(q1.ap(), q1.ap(), q2.ap())
                dr(eng)
                eng.reciprocal(q2.ap(), q1.ap())
                dr(eng)
                eng.tensor_mul(ff.ap(), sq.ap(), q2.ap())
                dr(eng)
                fb = ap3(ff, 0, [(1, O), (0, C)])
                eng.tensor_mul(out, src, fb).then_inc(outsem, 1)

            # phase 1 evac share
            for g in range(G):
                if _evac_owner(g) != 'D':
                    continue
                eng.wait_ge(S.pgsem, g + 1)
                eng.tensor_copy(ap3(u, g * 1024, [(1, 1024)]),
                                pg[g % 3].ap()).then_inc(S.evsemD, 1)
            # boundary-0 squash -> vrep[0:8]
            squash_std(0, ps, vrep.ap()[0:NB, :], S.v16sem)
            # routing: agr(k) then softmax(k-1) to hide ACT exp latency
            for t in range(2):
                for k in range(CH):
                    if t == 0:
                        glast = (k + 1) * CG - 1
                        for own, sem in (('A', S.evsemA), ('D', S.evsemD),
                                         ('P', S.evsemP)):
                            cnt = _evac_cnt(own, glast)
                            if cnt:
                                eng.wait_ge(sem, cnt)
                    if k == 0:
                        eng.wait_ge(S.vrsem, t + 1)
                    if not _is_pool(k):
                        agr_block(eng, t, k, tmpa, Ltmp, S.Lsem_d)
                    if k > 1:
                        softmax_diag(t, k - 2)
                softmax_diag(t, CH - 2)
                softmax_diag(t, CH - 1)
                if t == 0:
                    squash_std(1, ps, vrep.ap()[0:NB, :], S.v16sem)
                else:
                    squash_std(2, ps, vf.ap(), S.vfsem)

        # ---------------- GpSimd (pool) ----------------
        @block.gpsimd
        def _(eng):
            # phase 1: SWDGE WX loads for slots 4,5 + const loads
            for g in range(G):
                s = g % NWX
                if s >= 4:
                    if g >= NWX:
                        eng.wait_ge(S.wxfree, g - NWX + 1)
                    eng.dma_start(wxv(s, 0, 1152), WX.ap()[g]) \
                       .then_inc(wxsems[s], 16)
                    if g == 47:
                        eng.dma_start(eye.ap(), EYE.ap()).then_inc(S.d0c, 16)
                        eng.dma_start(reps.ap(), REPS.ap()) \
                           .then_inc(S.d0c, 16)
                        eng.dma_start(eyef.ap(), EYEF.ap()) \
                           .then_inc(S.d0c, 16)
                        eng.dma_start(mask.ap(), MASK.ap()) \
                           .then_inc(S.d0c, 16)
            # routing: agr mult+tree for pool chunks
            for t in range(2):
                for k in POOL_CHUNKS:
                    if t == 0:
                        glast = (k + 1) * CG - 1
                        for own, sem in (('A', S.evsemA), ('D', S.evsemD),
                                         ('P', S.evsemP)):
                            cnt = _evac_cnt(own, glast)
                            if cnt:
                                eng.wait_ge(sem, cnt)
                    eng.wait_ge(S.vrsem, t + 1)
                    agr_block(eng, t, k, tmpp, Ltmp_p, S.Lsem_p)

    return nc


def _preprocess(x, W):
    """Host-side repack (fp16 casts + layout) -> per-core input maps."""
    f16 = np.float16
    # W tiles: [g, (isub*8+d), (o*16+c)]
    Wt = np.ascontiguousarray(
        W.reshape(G, ISUB, O, D, C).transpose(0, 1, 3, 2, 4)
        .reshape(G, 128, 1024)).astype(f16)
    eyem = np.eye(128).astype(f16)
    # REPS[b, (isub,b')] = 1 if b == b' (replicates v16 over isub via PE)
    repsm = np.zeros((NB, 128), f16)
    for col in range(128):
        repsm[col % NB, col] = 1.0
    # MASK[(isub,b), (b', o)] = 1 if b==b'
    maskm = np.zeros((128, NB * O), f16)
    for p in range(128):
        b = p % NB
        maskm[p, b * O:(b + 1) * O] = 1.0
    in_maps = []
    for core in range(NCORES):
        xc = x[core * NB:(core + 1) * NB]            # [8, 1152, 8]
        xr = xc.reshape(NB, G, ISUB, D)              # (b, g, isub, d)
        xbd = np.zeros((G, 128, 128), f16)
        for isub in range(ISUB):
            xbd[:, isub * D:(isub + 1) * D, isub * NB:(isub + 1) * NB] = \
                xr[:, :, isub, :].transpose(1, 2, 0)  # (g, d, b)
        WXc = np.concatenate([Wt, xbd], axis=2)      # [72, 128, 1152]
        xdn = np.ascontiguousarray(
            (xr / 64.0).transpose(2, 3, 1, 0).reshape(128, G * NB)).astype(f16)
        in_maps.append({'WX': WXc, 'XDN': xdn, 'MASK': maskm, 'EYE': eyem,
                        'REPS': repsm, 'EYEF': eyem.astype(np.float32)})
    return in_maps


def _postprocess(results):
    out = np.empty((NCORES * NB, O, C), np.float32)
    for core in range(NCORES):
        vo = results[core]['VOUT']                   # [8, 1024] = (o, c)
        out[core * NB:(core + 1) * NB] = vo.reshape(NB, O, C)
    return out


def kernel(x, W):
    from concourse.bass_utils import run_bass_kernel_spmd
    x = np.asarray(x, np.float32)
    W = np.asarray(W, np.float32)
    if 'nc' not in _cache:
        _cache['nc'] = _build_program(paranoid=True)
    in_maps = _preprocess(x, W)
    res = run_bass_kernel_spmd(_cache['nc'], in_maps,
                               core_ids=list(range(NCORES)))
    return _postprocess(res.results)


def kernel_sim(x, W, core=0):
    """CoreSim single-core check: returns v for that core's 8 batch rows."""
    from concourse import bass_interp
    x = np.asarray(x, np.float32)
    W = np.asarray(W, np.float32)
    if 'nc_sim' not in _cache:
        _cache['nc_sim'] = _build_program(paranoid=True)
    in_maps = _preprocess(x, W)
    sim = bass_interp.CoreSim(_cache['nc_sim'])
    for name, arr in in_maps[core].items():
        sim.tensor(name)[:] = arr
    sim.simulate()
    vo = np.asarray(sim.tensor('VOUT'))
    return vo.reshape(NB, O, C)
